# revision 8
# baseline (speedup 1.0000x reference)
"""Trainium2 Bass kernel for nn_DataAugment (point-cloud augment).

reference semantics (per sample b):
    signs = flip sign on coord flip_axes[b] (if < 3)
    x  = batch_data * signs * scales[b]
    R  = rot-y by angle_table[rotated_gt[b]]
    c  = mean(x, axis=points)
    out = (x - c) @ R + c + noise
      == batch_data @ A + t + noise
    where A = diag(signs*scale) @ R (host-computable),
          t = (mean(batch_data) * signs*scale) @ (I - R)  (needs device mean).

Rot-y mixes only x/z:
    out_x = a*X + b*Z + t_x + n_x
    out_y = ey*Y + n_y
    out_z = g*X + d*Z + t_z + n_z
The device computes the four products on ScalarE (Identity w/ per-sample
scale column), harvesting per-partition sums via accum_out; a ones-matmul
on TensorE reduces partitions; tensor_tensor_reduce forms t; a second
ones-matmul broadcasts t back to a [128,2] column pair used as ACT bias.

Sharding: pure data parallel, batch dim 64 -> 8 cores x 8 samples.
"""

import os
import sys

import numpy as np

for _p in ("/opt/trn_rl_repo",):
    if os.path.isdir(_p) and _p not in sys.path:
        sys.path.insert(0, _p)

B, N = 64, 131072
NCORES = 8
BPC = B // NCORES      # samples per core
P = 128                # SBUF partitions
NPP = N // P           # points per partition (1024)
NBINS = 8
CSTR = 16              # coef columns per sample

# coef column layout (per sample block of CSTR):
# 0:a 1:b 2:g 3:d 4:ey 5..8:Cx[4] 9..12:Cz[4] 13..15:pad
_NC_CACHE = {}
LAST_RESULT = None


def _split_multi_waits(nc, mybir):
    """The walrus build in this container encodes at most ONE inline sync
    wait per instruction ("Too many sync wait commands" otherwise). Tile
    freely attaches several. Rewrite: spill all but the last wait onto
    same-engine NoOps placed immediately before the instruction (waits
    gate instruction issue, so a preceding same-engine wait is
    equivalent). Extra sem updates (if any) spill onto trailing NoOps."""
    uid = [0]
    for fn in nc.m.functions:
        for blk in fn.blocks:
            old = list(blk.instructions)
            new = []
            changed = False
            for ins in old:
                si = ins.sync_info
                waits = list(si.on_wait) if (si is not None and si.on_wait) else []
                ups = list(si.on_update) if (si is not None and si.on_update) else []
                if len(waits) > 1:
                    for w in waits[:-1]:
                        uid[0] += 1
                        new.append(mybir.InstEventSemaphore(
                            name=f"WS-{uid[0]}",
                            engine=ins.engine,
                            ins=[], outs=[],
                            sync_info=mybir.SyncInfo(on_wait=[w], on_update=[]),
                        ))
                    si.on_wait = waits[-1:]
                    changed = True
                new.append(ins)
                if len(ups) > 1:
                    for u in ups[1:]:
                        uid[0] += 1
                        new.append(mybir.InstEventSemaphore(
                            name=f"US-{uid[0]}",
                            engine=ins.engine,
                            ins=[], outs=[],
                            sync_info=mybir.SyncInfo(on_wait=[], on_update=[u]),
                        ))
                    si.on_update = ups[:1]
                    changed = True
            if changed:
                blk.instructions = new


def _build_nc(split_waits=True):
    import concourse.bass as bass
    import concourse.mybir as mybir
    from concourse.tile import TileContext

    fp32 = mybir.dt.float32
    Ident = mybir.ActivationFunctionType.Identity
    ActCopy = mybir.ActivationFunctionType.Copy
    mult = mybir.AluOpType.mult
    add = mybir.AluOpType.add

    nc = bass.Bass()
    x_dram = nc.dram_tensor("x", [BPC, P, NPP, 3], fp32, kind="ExternalInput")
    n_dram = nc.dram_tensor("nz", [BPC, P, NPP, 3], fp32, kind="ExternalInput")
    c_dram = nc.dram_tensor("coefs", [P, BPC * CSTR], fp32, kind="ExternalInput")
    o_dram = nc.dram_tensor("o", [BPC, P, NPP, 3], fp32, kind="ExternalOutput")

    with TileContext(nc) as tc:
        with (
            tc.tile_pool(name="const", bufs=1) as constp,
            tc.tile_pool(name="data", bufs=3) as datap,
            tc.tile_pool(name="scratch", bufs=2) as scrp,
            tc.tile_pool(name="small", bufs=8) as smallp,
            tc.tile_pool(name="ps", bufs=4, space="PSUM") as psp,
        ):
            coefs = constp.tile([P, BPC * CSTR], fp32, tag="coefs")
            nc.sync.dma_start(out=coefs[:], in_=c_dram[:])
            ones_col = constp.tile([P, 1], fp32, tag="ones_col")
            nc.vector.memset(ones_col[:], 1.0)
            ones_row = constp.tile([1, P], fp32, tag="ones_row")
            nc.vector.memset(ones_row[:], 1.0)

            for b in range(BPC):
                base = b * CSTR

                def cc(j, b_=base):
                    return coefs[:, b_ + j : b_ + j + 1]

                T = datap.tile([P, NPP, 3], fp32, tag="T")
                nc.sync.dma_start(out=T[:], in_=x_dram[b])
                Nz = datap.tile([P, NPP, 3], fp32, tag="Nz")
                nc.sync.dma_start(out=Nz[:], in_=n_dram[b])
                O = datap.tile([P, NPP, 3], fp32, tag="O")

                X, Y, Z = T[:, :, 0], T[:, :, 1], T[:, :, 2]

                px = scrp.tile([P, NPP], fp32, tag="px")
                bzx = scrp.tile([P, NPP], fp32, tag="bzx")
                gx = scrp.tile([P, NPP], fp32, tag="gx")
                bzz = scrp.tile([P, NPP], fp32, tag="bzz")
                s1x = scrp.tile([P, NPP], fp32, tag="s1x")
                s1z = scrp.tile([P, NPP], fp32, tag="s1z")
                acc = smallp.tile([P, 4], fp32, tag="acc")

                # products (ScalarE), harvesting per-partition raw sums
                nc.scalar.activation(px[:], X, ActCopy, bias=0.0,
                                     scale=cc(0), accum_out=acc[:, 0:1])
                nc.scalar.activation(bzx[:], Z, ActCopy, bias=0.0,
                                     scale=cc(1), accum_out=acc[:, 1:2])
                nc.scalar.activation(gx[:], X, ActCopy, bias=0.0,
                                     scale=cc(2), accum_out=acc[:, 2:3])
                nc.scalar.activation(bzz[:], Z, ActCopy, bias=0.0,
                                     scale=cc(3), accum_out=acc[:, 3:4])

                # partition-reduce the 4 accums: S_row[1,4] = ones.T @ acc
                s_ps = psp.tile([1, 4], fp32, tag="s_ps")
                nc.tensor.matmul(s_ps[:], ones_col[:], acc[:], start=True, stop=True)
                s_sb = smallp.tile([1, 4], fp32, tag="s_sb")
                nc.vector.tensor_copy(out=s_sb[:], in_=s_ps[:])

                # t_x = dot(S, Cx), t_z = dot(S, Cz)   (host-chosen coefs)
                t_row = smallp.tile([1, 2], fp32, tag="t_row")
                prods = smallp.tile([1, 2, 4], fp32, tag="prods")
                nc.vector.tensor_mul(
                    out=prods[:, 0, :], in0=s_sb[:],
                    in1=coefs[0:1, base + 5 : base + 9])
                nc.vector.tensor_mul(
                    out=prods[:, 1, :], in0=s_sb[:],
                    in1=coefs[0:1, base + 9 : base + 13])
                nc.vector.tensor_reduce(
                    out=t_row[:], in_=prods[:],
                    axis=mybir.AxisListType.X, op=add)

                # broadcast t to all partitions: [128,2] = ones_row.T @ t_row
                t_ps = psp.tile([P, 2], fp32, tag="t_ps")
                nc.tensor.matmul(t_ps[:], ones_row[:], t_row[:], start=True, stop=True)
                t_sb = smallp.tile([P, 2], fp32, tag="t_sb")
                nc.vector.tensor_copy(out=t_sb[:], in_=t_ps[:])

                # combine (VectorE adds + ScalarE bias-adds)
                nc.vector.tensor_add(out=s1x[:], in0=px[:], in1=bzx[:])
                nc.vector.tensor_add(out=s1z[:], in0=gx[:], in1=bzz[:])
                nc.scalar.activation(O[:, :, 0], s1x[:], Ident,
                                     bias=t_sb[:, 0:1], scale=1.0)
                nc.scalar.activation(O[:, :, 2], s1z[:], Ident,
                                     bias=t_sb[:, 1:2], scale=1.0)
                nc.scalar.activation(O[:, :, 1], Y, ActCopy, bias=0.0,
                                     scale=cc(4))

                # noise adds (in place on O)
                nc.vector.tensor_add(out=O[:, :, 0], in0=O[:, :, 0], in1=Nz[:, :, 0])
                nc.vector.tensor_add(out=O[:, :, 1], in0=O[:, :, 1], in1=Nz[:, :, 1])
                nc.vector.tensor_add(out=O[:, :, 2], in0=O[:, :, 2], in1=Nz[:, :, 2])

                nc.sync.dma_start(out=o_dram[b], in_=O[:])

    if split_waits:
        _split_multi_waits(nc, mybir)
    return nc


def _get_nc():
    if "nc" not in _NC_CACHE:
        _NC_CACHE["nc"] = _build_nc()
    return _NC_CACHE["nc"]


def _host_coefs(flip_axes, scales, rotated_gt):
    """Per-shard [128, BPC*CSTR] coefficient block (rows identical)."""
    bpc = len(scales)
    out = np.zeros((bpc, CSTR), dtype=np.float64)
    for i in range(bpc):
        f = int(flip_axes[i])
        sc = float(scales[i])
        k = int(rotated_gt[i])
        s3 = np.array([sc, sc, sc])
        if f < 3:
            s3[f] = -sc
        th = k * (2.0 * np.pi / NBINS) - np.pi / 4.0
        c, s = np.cos(th), np.sin(th)
        a = s3[0] * c          # X -> out_x
        bb = -s3[2] * s        # Z -> out_x
        g = s3[0] * s          # X -> out_z
        d = s3[2] * c          # Z -> out_z
        out[i, 0:5] = (a, bb, g, d, s3[1])
        # t_x = u_x*Sx/N + v_x*Sz/N ; t_z = u_z*Sx/N + v_z*Sz/N
        u_x = s3[0] * (1.0 - c)
        v_x = s3[2] * s
        u_z = -s3[0] * s
        v_z = s3[2] * (1.0 - c)
        # device accums: A = (a*Sx, bb*Sz, g*Sx, d*Sz)
        cx = np.zeros(4)
        cz = np.zeros(4)
        if abs(c) >= abs(s):
            cx[0] = u_x / (N * a)
            cz[0] = u_z / (N * a)
            cx[3] = v_x / (N * d)
            cz[3] = v_z / (N * d)
        else:
            cx[2] = u_x / (N * g)
            cz[2] = u_z / (N * g)
            cx[1] = v_x / (N * bb)
            cz[1] = v_z / (N * bb)
        out[i, 5:9] = cx
        out[i, 9:13] = cz
    flat = out.reshape(-1).astype(np.float32)
    return np.ascontiguousarray(np.broadcast_to(flat, (P, flat.size)))


def kernel(batch_data, flip_axes, scales, rotated_gt, noise):
    global LAST_RESULT
    from concourse.bass_utils import run_bass_kernel_spmd

    batch_data = np.ascontiguousarray(np.asarray(batch_data, dtype=np.float32))
    noise = np.ascontiguousarray(np.asarray(noise, dtype=np.float32))
    flip_np = np.asarray(flip_axes)
    scales_np = np.asarray(scales, dtype=np.float64)
    rot_np = np.asarray(rotated_gt)

    nc = _get_nc()

    in_maps = []
    for i in range(NCORES):
        sl = slice(i * BPC, (i + 1) * BPC)
        in_maps.append({
            "x": batch_data[sl].reshape(BPC, P, NPP, 3),
            "nz": noise[sl].reshape(BPC, P, NPP, 3),
            "coefs": _host_coefs(flip_np[sl], scales_np[sl], rot_np[sl]),
        })

    res = run_bass_kernel_spmd(nc, in_maps, core_ids=list(range(NCORES)))
    LAST_RESULT = res
    out = np.concatenate(
        [r["o"].reshape(BPC, N, 3) for r in res.results], axis=0
    )
    return (out, np.asarray(rotated_gt))


# revision 13
# speedup vs baseline: 1.0620x; 1.0620x over previous
"""Trainium2 Bass kernel for nn_DataAugment (point-cloud augment).

reference semantics (per sample b):
    signs = flip sign on coord flip_axes[b] (if < 3)
    x  = batch_data * signs * scales[b]
    R  = rot-y by angle_table[rotated_gt[b]]
    c  = mean(x, axis=points)
    out = (x - c) @ R + c + noise
      == batch_data @ A + t + noise
    where A = diag(signs*scale) @ R (host-computable),
          t = (mean(batch_data) * signs*scale) @ (I - R)  (needs device mean).

Rot-y mixes only x/z:
    out_x = a*X + b*Z + t_x + n_x
    out_y = ey*Y + n_y
    out_z = g*X + d*Z + t_z + n_z
The device computes the four products on ScalarE (Identity w/ per-sample
scale column), harvesting per-partition sums via accum_out; a ones-matmul
on TensorE reduces partitions; tensor_tensor_reduce forms t; a second
ones-matmul broadcasts t back to a [128,2] column pair used as ACT bias.

Sharding: pure data parallel, batch dim 64 -> 8 cores x 8 samples.
"""

import os
import sys

import numpy as np

for _p in ("/opt/trn_rl_repo",):
    if os.path.isdir(_p) and _p not in sys.path:
        sys.path.insert(0, _p)

B, N = 64, 131072
NCORES = 8
BPC = B // NCORES      # samples per core
P = 128                # SBUF partitions
NPP = N // P           # points per partition (1024)
NBINS = 8
CSTR = 16              # coef columns per sample

# coef column layout (per sample block of CSTR):
# 0:a 1:b 2:g 3:d 4:ey 5..8:Cx[4] 9..12:Cz[4] 13..15:pad
_NC_CACHE = {}
LAST_RESULT = None


def _split_multi_waits(nc, mybir):
    """The walrus build in this container encodes at most ONE inline sync
    wait per instruction ("Too many sync wait commands" otherwise). Tile
    freely attaches several. Rewrite: spill all but the last wait onto
    same-engine NoOps placed immediately before the instruction (waits
    gate instruction issue, so a preceding same-engine wait is
    equivalent). Extra sem updates (if any) spill onto trailing NoOps."""
    uid = [0]
    for fn in nc.m.functions:
        for blk in fn.blocks:
            old = list(blk.instructions)
            new = []
            changed = False
            for ins in old:
                si = ins.sync_info
                waits = list(si.on_wait) if (si is not None and si.on_wait) else []
                ups = list(si.on_update) if (si is not None and si.on_update) else []
                if len(waits) > 1:
                    for w in waits[:-1]:
                        uid[0] += 1
                        new.append(mybir.InstEventSemaphore(
                            name=f"WS-{uid[0]}",
                            engine=ins.engine,
                            ins=[], outs=[],
                            sync_info=mybir.SyncInfo(on_wait=[w], on_update=[]),
                        ))
                    si.on_wait = waits[-1:]
                    changed = True
                new.append(ins)
                if len(ups) > 1:
                    for u in ups[1:]:
                        uid[0] += 1
                        new.append(mybir.InstEventSemaphore(
                            name=f"US-{uid[0]}",
                            engine=ins.engine,
                            ins=[], outs=[],
                            sync_info=mybir.SyncInfo(on_wait=[], on_update=[u]),
                        ))
                    si.on_update = ups[:1]
                    changed = True
            if changed:
                blk.instructions = new


def _build_nc(split_waits=True):
    import concourse.bass as bass
    import concourse.mybir as mybir
    from concourse.tile import TileContext

    fp32 = mybir.dt.float32
    Ident = mybir.ActivationFunctionType.Identity
    ActCopy = mybir.ActivationFunctionType.Copy
    mult = mybir.AluOpType.mult
    add = mybir.AluOpType.add

    nc = bass.Bass()
    x_dram = nc.dram_tensor("x", [BPC, P, NPP, 3], fp32, kind="ExternalInput")
    n_dram = nc.dram_tensor("nz", [BPC, P, NPP, 3], fp32, kind="ExternalInput")
    c_dram = nc.dram_tensor("coefs", [P, BPC * CSTR], fp32, kind="ExternalInput")
    o_dram = nc.dram_tensor("o", [BPC, P, NPP, 3], fp32, kind="ExternalOutput")

    with TileContext(nc) as tc:
        with (
            tc.tile_pool(name="const", bufs=1) as constp,
            tc.tile_pool(name="data", bufs=4) as datap,
            tc.tile_pool(name="scratch", bufs=2) as scrp,
            tc.tile_pool(name="small", bufs=8) as smallp,
            tc.tile_pool(name="ps", bufs=4, space="PSUM") as psp,
        ):
            coefs = constp.tile([P, BPC * CSTR], fp32, tag="coefs")
            nc.sync.dma_start(out=coefs[:], in_=c_dram[:])
            ones_col = constp.tile([P, 1], fp32, tag="ones_col")
            nc.vector.memset(ones_col[:], 1.0)
            ones_row = constp.tile([1, P], fp32, tag="ones_row")
            nc.vector.memset(ones_row[:], 1.0)

            for b in range(BPC):
                base = b * CSTR

                def cc(j, b_=base):
                    return coefs[:, b_ + j : b_ + j + 1]

                T = datap.tile([P, NPP, 3], fp32, tag="T")
                nc.sync.dma_start(out=T[:], in_=x_dram[b])
                Nz = datap.tile([P, NPP, 3], fp32, tag="Nz")
                nc.sync.dma_start(out=Nz[:], in_=n_dram[b])
                O = datap.tile([P, NPP, 3], fp32, tag="O")

                X, Y, Z = T[:, :, 0], T[:, :, 1], T[:, :, 2]

                px = scrp.tile([P, NPP], fp32, tag="px")
                bzx = scrp.tile([P, NPP], fp32, tag="bzx")
                gx = scrp.tile([P, NPP], fp32, tag="gx")
                bzz = scrp.tile([P, NPP], fp32, tag="bzz")
                s1x = scrp.tile([P, NPP], fp32, tag="s1x")
                s1z = scrp.tile([P, NPP], fp32, tag="s1z")
                acc = smallp.tile([P, 4], fp32, tag="acc")

                # products (ScalarE), harvesting per-partition raw sums
                nc.scalar.activation(px[:], X, ActCopy, bias=0.0,
                                     scale=cc(0), accum_out=acc[:, 0:1])
                nc.scalar.activation(bzx[:], Z, ActCopy, bias=0.0,
                                     scale=cc(1), accum_out=acc[:, 1:2])
                nc.scalar.activation(gx[:], X, ActCopy, bias=0.0,
                                     scale=cc(2), accum_out=acc[:, 2:3])
                nc.scalar.activation(bzz[:], Z, ActCopy, bias=0.0,
                                     scale=cc(3), accum_out=acc[:, 3:4])

                # partition-reduce the 4 accums: S_row[1,4] = ones.T @ acc
                s_ps = psp.tile([1, 4], fp32, tag="s_ps")
                nc.tensor.matmul(s_ps[:], ones_col[:], acc[:], start=True, stop=True)
                s_sb = smallp.tile([1, 4], fp32, tag="s_sb")
                nc.vector.tensor_copy(out=s_sb[:], in_=s_ps[:])

                # t_x = dot(S, Cx), t_z = dot(S, Cz)   (host-chosen coefs)
                t_row = smallp.tile([1, 2], fp32, tag="t_row")
                prods = smallp.tile([1, 2, 4], fp32, tag="prods")
                nc.vector.tensor_mul(
                    out=prods[:, 0, :], in0=s_sb[:],
                    in1=coefs[0:1, base + 5 : base + 9])
                nc.vector.tensor_mul(
                    out=prods[:, 1, :], in0=s_sb[:],
                    in1=coefs[0:1, base + 9 : base + 13])
                nc.vector.tensor_reduce(
                    out=t_row[:], in_=prods[:],
                    axis=mybir.AxisListType.X, op=add)

                # broadcast t to all partitions: [128,2] = ones_row.T @ t_row
                t_ps = psp.tile([P, 2], fp32, tag="t_ps")
                nc.tensor.matmul(t_ps[:], ones_row[:], t_row[:], start=True, stop=True)
                t_sb = smallp.tile([P, 2], fp32, tag="t_sb")
                nc.vector.tensor_copy(out=t_sb[:], in_=t_ps[:])

                # combine: sums on DVE, t-biased stores on ACT, y on DVE
                nc.vector.tensor_add(out=s1x[:], in0=px[:], in1=bzx[:])
                nc.vector.tensor_add(out=s1z[:], in0=gx[:], in1=bzz[:])
                nc.scalar.activation(O[:, :, 0], s1x[:], Ident,
                                     bias=t_sb[:, 0:1], scale=1.0)
                nc.scalar.activation(O[:, :, 2], s1z[:], Ident,
                                     bias=t_sb[:, 1:2], scale=1.0)
                nc.vector.tensor_scalar(O[:, :, 1], Y, cc(4), 0.0, mult, add)

                # noise adds (in place on O)
                nc.vector.tensor_add(out=O[:, :, 0], in0=O[:, :, 0], in1=Nz[:, :, 0])
                nc.vector.tensor_add(out=O[:, :, 1], in0=O[:, :, 1], in1=Nz[:, :, 1])
                nc.vector.tensor_add(out=O[:, :, 2], in0=O[:, :, 2], in1=Nz[:, :, 2])

                nc.sync.dma_start(out=o_dram[b], in_=O[:])

    if split_waits:
        _split_multi_waits(nc, mybir)
    return nc


def _get_nc():
    if "nc" not in _NC_CACHE:
        _NC_CACHE["nc"] = _build_nc()
    return _NC_CACHE["nc"]


def _host_coefs(flip_axes, scales, rotated_gt):
    """Per-shard [128, BPC*CSTR] coefficient block (rows identical)."""
    bpc = len(scales)
    out = np.zeros((bpc, CSTR), dtype=np.float64)
    for i in range(bpc):
        f = int(flip_axes[i])
        sc = float(scales[i])
        k = int(rotated_gt[i])
        s3 = np.array([sc, sc, sc])
        if f < 3:
            s3[f] = -sc
        th = k * (2.0 * np.pi / NBINS) - np.pi / 4.0
        c, s = np.cos(th), np.sin(th)
        a = s3[0] * c          # X -> out_x
        bb = -s3[2] * s        # Z -> out_x
        g = s3[0] * s          # X -> out_z
        d = s3[2] * c          # Z -> out_z
        out[i, 0:5] = (a, bb, g, d, s3[1])
        # t_x = u_x*Sx/N + v_x*Sz/N ; t_z = u_z*Sx/N + v_z*Sz/N
        u_x = s3[0] * (1.0 - c)
        v_x = s3[2] * s
        u_z = -s3[0] * s
        v_z = s3[2] * (1.0 - c)
        # device accums: A = (a*Sx, bb*Sz, g*Sx, d*Sz)
        cx = np.zeros(4)
        cz = np.zeros(4)
        if abs(c) >= abs(s):
            cx[0] = u_x / (N * a)
            cz[0] = u_z / (N * a)
            cx[3] = v_x / (N * d)
            cz[3] = v_z / (N * d)
        else:
            cx[2] = u_x / (N * g)
            cz[2] = u_z / (N * g)
            cx[1] = v_x / (N * bb)
            cz[1] = v_z / (N * bb)
        out[i, 5:9] = cx
        out[i, 9:13] = cz
    flat = out.reshape(-1).astype(np.float32)
    return np.ascontiguousarray(np.broadcast_to(flat, (P, flat.size)))


def kernel(batch_data, flip_axes, scales, rotated_gt, noise):
    global LAST_RESULT
    from concourse.bass_utils import run_bass_kernel_spmd

    batch_data = np.ascontiguousarray(np.asarray(batch_data, dtype=np.float32))
    noise = np.ascontiguousarray(np.asarray(noise, dtype=np.float32))
    flip_np = np.asarray(flip_axes)
    scales_np = np.asarray(scales, dtype=np.float64)
    rot_np = np.asarray(rotated_gt)

    nc = _get_nc()

    in_maps = []
    for i in range(NCORES):
        sl = slice(i * BPC, (i + 1) * BPC)
        in_maps.append({
            "x": batch_data[sl].reshape(BPC, P, NPP, 3),
            "nz": noise[sl].reshape(BPC, P, NPP, 3),
            "coefs": _host_coefs(flip_np[sl], scales_np[sl], rot_np[sl]),
        })

    res = run_bass_kernel_spmd(nc, in_maps, core_ids=list(range(NCORES)))
    LAST_RESULT = res
    out = np.concatenate(
        [r["o"].reshape(BPC, N, 3) for r in res.results], axis=0
    )
    return (out, np.asarray(rotated_gt))


# revision 14
# speedup vs baseline: 1.1514x; 1.0842x over previous
"""Trainium2 Bass kernel for nn_DataAugment (point-cloud augment).

reference semantics (per sample b):
    signs = flip sign on coord flip_axes[b] (if < 3)
    x  = batch_data * signs * scales[b]
    R  = rot-y by angle_table[rotated_gt[b]]
    c  = mean(x, axis=points)
    out = (x - c) @ R + c + noise
      == batch_data @ A + t + noise
    where A = diag(signs*scale) @ R (host-computable),
          t = (mean(batch_data) * signs*scale) @ (I - R)  (needs device mean).

Rot-y mixes only x/z:
    out_x = a*X + b*Z + t_x + n_x
    out_y = ey*Y + n_y
    out_z = g*X + d*Z + t_z + n_z
The device computes the four products on ScalarE (Identity w/ per-sample
scale column), harvesting per-partition sums via accum_out; a ones-matmul
on TensorE reduces partitions; tensor_tensor_reduce forms t; a second
ones-matmul broadcasts t back to a [128,2] column pair used as ACT bias.

Sharding: pure data parallel, batch dim 64 -> 8 cores x 8 samples.
"""

import os
import sys

import numpy as np

for _p in ("/opt/trn_rl_repo",):
    if os.path.isdir(_p) and _p not in sys.path:
        sys.path.insert(0, _p)

B, N = 64, 131072
NCORES = 8
BPC = B // NCORES      # samples per core
P = 128                # SBUF partitions
NPP = N // P           # points per partition (1024)
NBINS = 8
CSTR = 16              # coef columns per sample

# coef column layout (per sample block of CSTR):
# 0:a 1:b 2:g 3:d 4:ey 5..8:Cx[4] 9..12:Cz[4] 13..15:pad
_NC_CACHE = {}
LAST_RESULT = None


def _split_multi_waits(nc, mybir):
    """The walrus build in this container encodes at most ONE inline sync
    wait per instruction ("Too many sync wait commands" otherwise). Tile
    freely attaches several. Rewrite: spill all but the last wait onto
    same-engine NoOps placed immediately before the instruction (waits
    gate instruction issue, so a preceding same-engine wait is
    equivalent). Extra sem updates (if any) spill onto trailing NoOps."""
    uid = [0]
    for fn in nc.m.functions:
        for blk in fn.blocks:
            old = list(blk.instructions)
            new = []
            changed = False
            for ins in old:
                si = ins.sync_info
                waits = list(si.on_wait) if (si is not None and si.on_wait) else []
                ups = list(si.on_update) if (si is not None and si.on_update) else []
                if len(waits) > 1:
                    for w in waits[:-1]:
                        uid[0] += 1
                        new.append(mybir.InstEventSemaphore(
                            name=f"WS-{uid[0]}",
                            engine=ins.engine,
                            ins=[], outs=[],
                            sync_info=mybir.SyncInfo(on_wait=[w], on_update=[]),
                        ))
                    si.on_wait = waits[-1:]
                    changed = True
                new.append(ins)
                if len(ups) > 1:
                    for u in ups[1:]:
                        uid[0] += 1
                        new.append(mybir.InstEventSemaphore(
                            name=f"US-{uid[0]}",
                            engine=ins.engine,
                            ins=[], outs=[],
                            sync_info=mybir.SyncInfo(on_wait=[], on_update=[u]),
                        ))
                    si.on_update = ups[:1]
                    changed = True
            if changed:
                blk.instructions = new


def _build_nc(split_waits=True):
    import concourse.bass as bass
    import concourse.mybir as mybir
    from concourse.tile import TileContext

    fp32 = mybir.dt.float32
    Ident = mybir.ActivationFunctionType.Identity
    ActCopy = mybir.ActivationFunctionType.Copy
    mult = mybir.AluOpType.mult
    add = mybir.AluOpType.add

    nc = bass.Bass()
    x_dram = nc.dram_tensor("x", [BPC, P, NPP, 3], fp32, kind="ExternalInput")
    n_dram = nc.dram_tensor("nz", [BPC, P, NPP, 3], fp32, kind="ExternalInput")
    c_dram = nc.dram_tensor("coefs", [P, BPC * CSTR], fp32, kind="ExternalInput")
    o_dram = nc.dram_tensor("o", [BPC, P, NPP, 3], fp32, kind="ExternalOutput")

    with TileContext(nc) as tc:
        with (
            tc.tile_pool(name="const", bufs=1) as constp,
            tc.tile_pool(name="data", bufs=4) as datap,
            tc.tile_pool(name="scratch", bufs=2) as scrp,
            tc.tile_pool(name="small", bufs=8) as smallp,
            tc.tile_pool(name="ps", bufs=4, space="PSUM") as psp,
        ):
            coefs = constp.tile([P, BPC * CSTR], fp32, tag="coefs")
            nc.sync.dma_start(out=coefs[:], in_=c_dram[:])
            ones_mat = constp.tile([P, P], fp32, tag="ones_mat")
            nc.vector.memset(ones_mat[:], 1.0)

            for b in range(BPC):
                base = b * CSTR

                def cc(j, b_=base):
                    return coefs[:, b_ + j : b_ + j + 1]

                T = datap.tile([P, NPP, 3], fp32, tag="T")
                nc.sync.dma_start(out=T[:], in_=x_dram[b])
                Nz = datap.tile([P, NPP, 3], fp32, tag="Nz")
                nc.sync.dma_start(out=Nz[:], in_=n_dram[b])
                O = datap.tile([P, NPP, 3], fp32, tag="O")

                X, Y, Z = T[:, :, 0], T[:, :, 1], T[:, :, 2]

                px = scrp.tile([P, NPP], fp32, tag="px")
                bzx = scrp.tile([P, NPP], fp32, tag="bzx")
                gx = scrp.tile([P, NPP], fp32, tag="gx")
                bzz = scrp.tile([P, NPP], fp32, tag="bzz")
                s1x = scrp.tile([P, NPP], fp32, tag="s1x")
                s1z = scrp.tile([P, NPP], fp32, tag="s1z")
                acc = smallp.tile([P, 4], fp32, tag="acc")

                # products (ScalarE), harvesting per-partition raw sums
                nc.scalar.activation(px[:], X, ActCopy, bias=0.0,
                                     scale=cc(0), accum_out=acc[:, 0:1])
                nc.scalar.activation(bzx[:], Z, ActCopy, bias=0.0,
                                     scale=cc(1), accum_out=acc[:, 1:2])
                nc.scalar.activation(gx[:], X, ActCopy, bias=0.0,
                                     scale=cc(2), accum_out=acc[:, 2:3])
                nc.scalar.activation(bzz[:], Z, ActCopy, bias=0.0,
                                     scale=cc(3), accum_out=acc[:, 3:4])

                # per-partition partial dots d = [acc.Cx, acc.Cz], then one
                # ones-matmul does partition-sum + broadcast in a single shot
                dvec = smallp.tile([P, 2], fp32, tag="dvec")
                tmp4 = smallp.tile([P, 2, 4], fp32, tag="tmp4")
                nc.vector.tensor_mul(
                    out=tmp4[:, 0, :], in0=acc[:],
                    in1=coefs[:, base + 5 : base + 9])
                nc.vector.tensor_mul(
                    out=tmp4[:, 1, :], in0=acc[:],
                    in1=coefs[:, base + 9 : base + 13])
                nc.vector.tensor_reduce(
                    out=dvec[:], in_=tmp4[:],
                    axis=mybir.AxisListType.X, op=add)

                t_ps = psp.tile([P, 2], fp32, tag="t_ps")
                nc.tensor.matmul(t_ps[:], ones_mat[:], dvec[:], start=True, stop=True)
                t_sb = smallp.tile([P, 2], fp32, tag="t_sb")
                nc.vector.tensor_copy(out=t_sb[:], in_=t_ps[:])

                # combine (all on DVE): sum, +noise, then +t via ptr scalar
                nc.vector.tensor_add(out=s1x[:], in0=px[:], in1=bzx[:])
                nc.vector.tensor_add(out=s1z[:], in0=gx[:], in1=bzz[:])
                nc.vector.tensor_add(out=s1x[:], in0=s1x[:], in1=Nz[:, :, 0])
                nc.vector.tensor_add(out=s1z[:], in0=s1z[:], in1=Nz[:, :, 2])
                nc.vector.tensor_scalar(O[:, :, 0], s1x[:], t_sb[:, 0:1], None, add)
                nc.vector.tensor_scalar(O[:, :, 2], s1z[:], t_sb[:, 1:2], None, add)
                nc.vector.tensor_scalar(O[:, :, 1], Y, cc(4), 0.0, mult, add)
                nc.vector.tensor_add(out=O[:, :, 1], in0=O[:, :, 1], in1=Nz[:, :, 1])

                nc.sync.dma_start(out=o_dram[b], in_=O[:])

    if split_waits:
        _split_multi_waits(nc, mybir)
    return nc


def _get_nc():
    if "nc" not in _NC_CACHE:
        _NC_CACHE["nc"] = _build_nc()
    return _NC_CACHE["nc"]


def _host_coefs(flip_axes, scales, rotated_gt):
    """Per-shard [128, BPC*CSTR] coefficient block (rows identical)."""
    bpc = len(scales)
    out = np.zeros((bpc, CSTR), dtype=np.float64)
    for i in range(bpc):
        f = int(flip_axes[i])
        sc = float(scales[i])
        k = int(rotated_gt[i])
        s3 = np.array([sc, sc, sc])
        if f < 3:
            s3[f] = -sc
        th = k * (2.0 * np.pi / NBINS) - np.pi / 4.0
        c, s = np.cos(th), np.sin(th)
        a = s3[0] * c          # X -> out_x
        bb = -s3[2] * s        # Z -> out_x
        g = s3[0] * s          # X -> out_z
        d = s3[2] * c          # Z -> out_z
        out[i, 0:5] = (a, bb, g, d, s3[1])
        # t_x = u_x*Sx/N + v_x*Sz/N ; t_z = u_z*Sx/N + v_z*Sz/N
        u_x = s3[0] * (1.0 - c)
        v_x = s3[2] * s
        u_z = -s3[0] * s
        v_z = s3[2] * (1.0 - c)
        # device accums: A = (a*Sx, bb*Sz, g*Sx, d*Sz)
        cx = np.zeros(4)
        cz = np.zeros(4)
        if abs(c) >= abs(s):
            cx[0] = u_x / (N * a)
            cz[0] = u_z / (N * a)
            cx[3] = v_x / (N * d)
            cz[3] = v_z / (N * d)
        else:
            cx[2] = u_x / (N * g)
            cz[2] = u_z / (N * g)
            cx[1] = v_x / (N * bb)
            cz[1] = v_z / (N * bb)
        out[i, 5:9] = cx
        out[i, 9:13] = cz
    flat = out.reshape(-1).astype(np.float32)
    return np.ascontiguousarray(np.broadcast_to(flat, (P, flat.size)))


def kernel(batch_data, flip_axes, scales, rotated_gt, noise):
    global LAST_RESULT
    from concourse.bass_utils import run_bass_kernel_spmd

    batch_data = np.ascontiguousarray(np.asarray(batch_data, dtype=np.float32))
    noise = np.ascontiguousarray(np.asarray(noise, dtype=np.float32))
    flip_np = np.asarray(flip_axes)
    scales_np = np.asarray(scales, dtype=np.float64)
    rot_np = np.asarray(rotated_gt)

    nc = _get_nc()

    in_maps = []
    for i in range(NCORES):
        sl = slice(i * BPC, (i + 1) * BPC)
        in_maps.append({
            "x": batch_data[sl].reshape(BPC, P, NPP, 3),
            "nz": noise[sl].reshape(BPC, P, NPP, 3),
            "coefs": _host_coefs(flip_np[sl], scales_np[sl], rot_np[sl]),
        })

    res = run_bass_kernel_spmd(nc, in_maps, core_ids=list(range(NCORES)))
    LAST_RESULT = res
    out = np.concatenate(
        [r["o"].reshape(BPC, N, 3) for r in res.results], axis=0
    )
    return (out, np.asarray(rotated_gt))
